# revision 5
# baseline (speedup 1.0000x reference)
"""Trainium2 Bass kernel for DecomposedShiftNet, v2 (fp8 DoubleRow).

Sharding: 8 cores = 4 batch quarters x 2 position halves.
  core = g*4 + q : batch rows q*512..q*512+512, positions g*32..g*32+32.
Per core: BCC=512 batch cols, 32 positions; block = 1 position x 512 batch.

Per block (NB=512 free cols):
  h1   : 4x tensor_scalar (sp_c + pb[c,i], max 0) -> fp8, scaled 2^SH1
  h2   : 8 fp8 DoubleRow matmuls (K=512 as 2 k-pairs x 4 m-chunks) into
         two [128,2,512] psum tiles; evict Relu+bias -> fp8 (scale folded)
  lg   : 2 fp8 DoubleRow matmuls (M=64, no dup); Exp evict (scale arg) ->
         exp_t[0:64] bf16; DVE mult by abT -> exp_t[64:128]
  dn/v : one [3,NB] psum: dn_w (M=3: 2*ones_d | ones_n | 0) on exp_t K=128
         + 2 bf16 matmuls (0|0|w2c) on vh K=2x128; copy evict + 1 DMA to
         stash[j] ([sn,3,NB] segment accumulators)
  vh   : 2x tensor_scalar bf16 (DVE 4x mode)
Tail per 8-block segment: rd=recip_approx_fast(2d), pq=n*rd (TT),
  tgh=Tanh(0.5v+0.5b) [same act table as Exp/Relu], out=(tgh+1)*pq (STT)
  -> bf16, PE transposes -> obm [128,4,32] f32 -> one DMA out [512,32].
"""

import sys

import ml_dtypes
import numpy as np

for _p in ("/opt/trn_rl_repo",):
    if _p not in sys.path:
        sys.path.insert(0, _p)

import concourse.bacc as bacc
import concourse.bass as bass
import concourse.tile as tile
from concourse import bass_utils, mybir

F32 = mybir.dt.float32
F32R = mybir.dt.float32r
BF16 = mybir.dt.bfloat16
F8 = mybir.dt.float8e4
AF = mybir.ActivationFunctionType
OP = mybir.AluOpType
DR = mybir.MatmulPerfMode.DoubleRow

B, BITS, H = 2048, 64, 512
NCORES = 8
BCC = 512           # batch rows per core
NPOS = 32           # positions per core
NB = BCC            # free columns per block (1 position)
HV = H // 2         # validity hidden = 256
SEGS = [(0, 8), (8, 8), (16, 8), (24, 4), (28, 4)]
SEG_OF = {}
for _si, (_s0, _sn) in enumerate(SEGS):
    for _k in range(_sn):
        SEG_OF[_s0 + _k] = (_si, _k)

# power-of-2 scales, computed in make_in_maps (module-level so _emit's
# compiled program is scale-agnostic: scales only touch host-side data
# except the Exp scale which is baked in -> keep it a fixed constant).
SH1 = 7    # h1 fp8 = 2^SH1 * h1_true  (h1 max ~0.1 -> 12.8)
SW2 = 4    # w2 fp8 = 2^SW2 * w2       (h2 scale 2^11: h2 max ~0.06 -> 123)
SW3 = 6    # w3 fp8 = 2^SW3 * w3
SH2 = SH1 + SW2
EXP_SCALE = 2.0 ** (-(SH2 + SW3))


def to_f32r_np(a):
    u = np.ascontiguousarray(a, dtype=np.float32).view(np.uint32)
    r = (u + 0x7FF + ((u >> 12) & 1)) & np.uint32(0xFFFFF000)
    return r.view(np.float32)


def to_f8(a):
    a = np.clip(np.asarray(a, np.float32), -224.0, 224.0)
    return a.astype(ml_dtypes.float8_e4m3fn)


# name -> (shape, dtype)
_INPUTS = {
    "sbT": ((BITS, BCC), "f32r"),
    "abT": ((BITS, BCC), "bf16"),
    "ones64": ((BITS, 1), "f32r"),
    "ones1": ((1, BITS), "f32r"),
    "pb": ((128, 4, NPOS), "f32"),
    "vpb": ((128, 2, NPOS), "f32"),
    "wsd1": ((BITS, H), "f32r"),
    "wsd2": ((128, 4, H), "f32r"),
    "wsd3": ((128, 4, BITS), "f32r"),
    "sdb1": ((128, 4), "f32"),
    "sdb2": ((128, 4), "f32"),
    "sdb3": ((BITS, 1), "f32"),
    "wixb": ((BITS, H), "f32r"),
    "wvb": ((BITS, HV), "f32r"),
    "wix2": ((128, 4, H), "f8"),
    "b2s": ((128, 4), "f32"),
    "wix3": ((128, 4, BITS), "f8"),
    "ixb3": ((BITS, 1), "f32"),
    "dn_w": ((128, 32), "bf16"),
    "wv2x": ((128, 2, 32), "bf16"),
    "ident8": ((8, 8), "bf16"),
    "vb2t": ((8, 1), "f32"),
}
_DT = {"f32": F32, "f32r": F32R, "bf16": BF16, "f8": F8}


def _emit(nc, tc, I, out):
    import contextlib

    ctx = contextlib.ExitStack()
    with ctx:
        const = ctx.enter_context(tc.tile_pool(name="const", bufs=1))
        work = ctx.enter_context(tc.tile_pool(name="work", bufs=5))
        psH = ctx.enter_context(tc.tile_pool(name="psH", bufs=2, space="PSUM"))
        psL = ctx.enter_context(tc.tile_pool(name="psL", bufs=2, space="PSUM"))
        psD = ctx.enter_context(tc.tile_pool(name="psD", bufs=2, space="PSUM"))

        # ---------------- load inputs (round-robin DMA queues) ----------------
        T = {}
        qs = [nc.sync, nc.gpsimd]

        def load(name, qi):
            shape, code = _INPUTS[name]
            t = const.tile(list(shape), _DT[code], tag=name, name=name)
            qs[qi % len(qs)].dma_start(out=t, in_=I[name])
            T[name] = t

        def load_split(name):
            shape, code = _INPUTS[name]
            t = const.tile(list(shape), _DT[code], tag=name, name=name)
            h = shape[-1] // 2
            nc.sync.dma_start(out=t[..., 0:h], in_=I[name][..., 0:h])
            nc.gpsimd.dma_start(out=t[..., h:], in_=I[name][..., h:])
            T[name] = t

        load_split("sbT")
        load_split("wsd1")
        order = ["sdb1", "wsd2", "sdb2", "wsd3", "sdb3",
                 "ones64", "ones1", "wixb", "pb", "wix2", "b2s", "abT",
                 "wvb", "vpb", "wix3", "ixb3", "dn_w", "wv2x", "ident8",
                 "vb2t"]
        for qi, name in enumerate(order):
            load(name, qi)

        sbT, abT = T["sbT"], T["abT"]
        ones64, ones1 = T["ones64"], T["ones1"]
        pb, vpb = T["pb"], T["vpb"]
        wsd1, wsd2, wsd3 = T["wsd1"], T["wsd2"], T["wsd3"]
        sdb1, sdb2, sdb3 = T["sdb1"], T["sdb2"], T["sdb3"]
        wixb, wvb = T["wixb"], T["wvb"]
        wix2, b2s = T["wix2"], T["b2s"]
        wix3, ixb3 = T["wix3"], T["ixb3"]
        dn_w, wv2x, ident8 = T["dn_w"], T["wv2x"], T["ident8"]

        # ---------------- shift decoder MLP (f32r, N=512) ----------------
        hsd1 = const.tile([128, 4, BCC], F32R)
        for m in range(4):
            ps = psL.tile([128, BCC], F32, tag="lg", name="ps1")
            nc.tensor.matmul(ps, wsd1[:, m * 128:(m + 1) * 128], sbT,
                             start=True, stop=True)
            nc.scalar.activation(hsd1[:, m, :], ps, AF.Relu, bias=sdb1[:, m:m + 1])
        hsd2 = const.tile([128, 4, BCC], F32R)
        for m in range(4):
            ps = psL.tile([128, BCC], F32, tag="lg", name="ps2")
            for k in range(4):
                nc.tensor.matmul(ps, wsd2[:, k, m * 128:(m + 1) * 128],
                                 hsd1[:, k, :], start=(k == 0), stop=(k == 3))
            nc.scalar.activation(hsd2[:, m, :], ps, AF.Relu, bias=sdb2[:, m:m + 1])
        ps3 = psL.tile([64, BCC], F32, tag="lg", name="ps3")
        for k in range(4):
            nc.tensor.matmul(ps3, wsd3[:, k, :], hsd2[:, k, :],
                             start=(k == 0), stop=(k == 3))
        exp_sd = const.tile([64, BCC], F32R)
        nc.scalar.activation(exp_sd, ps3, AF.Exp, bias=sdb3)

        # softmax normalize via ones-matmul + approx reciprocal + K=1 bcast
        psd = psD.tile([1, BCC], F32, tag="dnv", name="psd")
        nc.tensor.matmul(psd, ones64, exp_sd, start=True, stop=True)
        rec_sd = const.tile([1, BCC], F32)
        nc.vector.reciprocal_approx_fast(out=rec_sd, in_=psd)
        rec_r = const.tile([1, BCC], F32R)
        nc.vector.tensor_copy(out=rec_r, in_=rec_sd)
        psb = psL.tile([64, BCC], F32, tag="lg", name="psb")
        nc.tensor.matmul(psb, ones1, rec_r, start=True, stop=True)
        psb_s = const.tile([64, BCC], F32R)
        nc.vector.tensor_copy(out=psb_s, in_=psb)
        shift_soft = const.tile([64, BCC], F32R)
        nc.vector.tensor_tensor(shift_soft, exp_sd, psb_s, OP.mult)

        # sp = 2^SH1 * shift_soft @ ix_w1[64:]  (wixb pre-scaled), bf16
        sp = const.tile([128, 4, BCC], BF16)
        for m in range(4):
            ps = psL.tile([128, BCC], F32, tag="lg", name="psp")
            nc.tensor.matmul(ps, wixb[:, m * 128:(m + 1) * 128], shift_soft,
                             start=True, stop=True)
            if m % 2 == 0:
                nc.scalar.activation(sp[:, m, :], ps, AF.Copy)
            else:
                nc.vector.tensor_copy(out=sp[:, m, :], in_=ps)
        vs = const.tile([128, 2, BCC], BF16)
        for m in range(2):
            ps = psL.tile([128, BCC], F32, tag="lg", name="psv")
            nc.tensor.matmul(ps, wvb[:, m * 128:(m + 1) * 128], shift_soft,
                             start=True, stop=True)
            if m == 0:
                nc.scalar.activation(vs[:, m, :], ps, AF.Copy)
            else:
                nc.vector.tensor_copy(out=vs[:, m, :], in_=ps)

        # ---------------- stash accumulators + output staging ----------------
        stash = [const.tile([sn, 3, NB], F32, tag=f"stash{i}", name=f"stash{i}")
                 for i, (_, sn) in enumerate(SEGS)]
        obm = const.tile([128, 4, NPOS], F32, tag="obm", name="obm")

        st = {}

        def stage_h1_h2(j):
            d = st[j] = {}
            h1 = work.tile([128, 4, NB], F8, tag="h1", name="h1")
            for c in range(4):
                nc.vector.tensor_scalar(h1[:, c, :], sp[:, c, :],
                                        pb[:, c, j:j + 1], 0.0, OP.add, OP.max)
            # h2: 3-deep ring of [128, 2, NB] psum tiles; 2 DR k-pairs per m
            psa = psH.tile([128, 2, NB], F32, tag="h2", name="h2a")
            psc = psH.tile([128, 2, NB], F32, tag="h2", name="h2b")
            for m in range(4):
                pt = psa if m < 2 else psc
                for kk in range(2):
                    nc.tensor.matmul(
                        pt[:, m % 2, :],
                        wix2[:, 2 * kk:2 * kk + 2, m * 128:(m + 1) * 128],
                        h1[:, 2 * kk:2 * kk + 2, :],
                        start=(kk == 0), stop=(kk == 1), perf_mode=DR)
            h2 = d["h2"] = work.tile([128, 4, NB], F8, tag="h2", name="h2")
            nc.scalar.activation(h2[:, 0:2, :], psa, AF.Relu, bias=b2s[:, 0:1])
            nc.scalar.activation(h2[:, 2:4, :], psc, AF.Relu, bias=b2s[:, 2:3])
            vh = d["vh"] = work.tile([128, 2, NB], BF16, tag="vh", name="vh")
            for c in range(2):
                nc.vector.tensor_scalar(vh[:, c, :], vs[:, c, :],
                                        vpb[:, c, j:j + 1], 0.0, OP.add, OP.max)

        def stage_logits(j):
            d = st[j]
            pl = psL.tile([64, NB], F32, tag="lg", name="lg")
            for kk in range(2):
                nc.tensor.matmul(
                    pl, wix3[:, 2 * kk:2 * kk + 2, :],
                    d["h2"][:, 2 * kk:2 * kk + 2, :],
                    start=(kk == 0), stop=(kk == 1), perf_mode=DR)
            exp_t = d["exp"] = work.tile([128, NB], BF16, tag="exp", name="exp")
            nc.scalar.activation(exp_t[0:64, :], pl, AF.Exp,
                                 bias=ixb3, scale=EXP_SCALE)
            nc.vector.tensor_tensor(exp_t[64:128, :], exp_t[0:64, :], abT, OP.mult)

        grp = {}

        def stage_reduce(j):
            d = st[j]
            r = j % 4
            if r == 0:
                grp["pdn"] = psD.tile([128, NB], F32, tag="dnv", name="dnv")
            pdn = grp["pdn"]
            nc.tensor.matmul(pdn[32 * r:32 * r + 32, :], dn_w, d["exp"],
                             start=True, stop=False, tile_position=(0, 32 * r))
            for c in range(2):
                nc.tensor.matmul(pdn[32 * r:32 * r + 32, :], wv2x[:, c, :],
                                 d["vh"][:, c, :], start=False, stop=(c == 1),
                                 tile_position=(0, 32 * r))
            if r == 3:
                row = work.tile([128, NB], F32, tag="dnvsb", name="dnvsb")
                nc.vector.tensor_copy(out=row, in_=pdn)
                j0 = j - 3
                for g in range(4):
                    si, jj = SEG_OF[j0 + g]
                    nc.sync.dma_start(out=stash[si][jj:jj + 1, :, :],
                                      in_=row[32 * g:32 * g + 3, :])
            del st[j]

        tails = {}

        def tail_compute(si):
            s0, sn = SEGS[si]
            stv = stash[si]
            rd = work.tile([sn, NB], F32, tag="tailrd", name="tailrd")
            nc.vector.reciprocal_approx_fast(out=rd, in_=stv[:, 0, :])
            pq = work.tile([sn, NB], F32, tag="tailpq", name="tailpq")
            nc.vector.tensor_tensor(pq, stv[:, 1, :], rd, OP.mult)
            tgh = work.tile([sn, NB], F32, tag="tailtg", name="tailtg")
            nc.scalar.activation(tgh, stv[:, 2, :], AF.Tanh,
                                 bias=T["vb2t"][0:sn, :], scale=0.5)
            outv = tails[si] = work.tile([sn, NB], BF16, tag="tailout", name="tailout")
            nc.vector.scalar_tensor_tensor(outv, tgh, 1.0, pq, OP.add, OP.mult)

        outr = out.rearrange("(c p) j -> p c j", c=4)

        def tail_transpose(si):
            s0, sn = SEGS[si]
            outv = tails.pop(si)
            pt = psL.tile([128, 4, sn], BF16, tag="lg", name="ptr")
            for c in range(4):
                nc.tensor.transpose(pt[:, c, :], outv[:, c * 128:(c + 1) * 128],
                                    ident8[0:sn, 0:sn])
            nc.vector.tensor_copy(out=obm[:, :, s0:s0 + sn], in_=pt)
            nc.sync.dma_start(out=outr[:, :, s0:s0 + sn],
                              in_=obm[:, :, s0:s0 + sn])

        for j in range(NPOS + 4):
            if j < NPOS:
                stage_h1_h2(j)
            if 4 <= j:
                stage_reduce(j - 4)
            if 2 <= j <= NPOS + 1:
                stage_logits(j - 2)
            if j in (12, 20, 28):
                tail_compute((j - 12) // 8)
            if j in (14, 22, 30):
                tail_transpose((j - 14) // 8)
            if j == 32:
                tail_compute(3)
            if j == 33:
                tail_transpose(3)
        tail_compute(4)
        tail_transpose(4)


def build_program():
    nc = bacc.Bacc("TRN2", target_bir_lowering=False, debug=False,
                   enable_asserts=False)
    I = {}
    for name, (shape, code) in _INPUTS.items():
        I[name] = nc.dram_tensor(name, list(shape), _DT[code],
                                 kind="ExternalInput").ap()
    out = nc.dram_tensor("out", [BCC, NPOS], F32, kind="ExternalOutput").ap()
    with tile.TileContext(nc) as tc:
        _emit(nc, tc, I, out)
    nc.compile()
    return nc


_NC = None


def _get_program():
    global _NC
    if _NC is None:
        _NC = build_program()
    return _NC


def make_in_maps(inputs):
    f = {k: np.ascontiguousarray(np.asarray(v, dtype=np.float32))
         for k, v in inputs.items()}
    r = to_f32r_np
    s1, s2, s3 = 2.0 ** SH1, 2.0 ** SW2, 2.0 ** SW3

    dn = np.zeros((128, 32), np.float32)
    dn[0:64, 0] = 2.0   # d*2 for the tail recip trick
    dn[64:128, 1] = 1.0
    wv2 = np.zeros((128, 2, 32), np.float32)
    wv2[:, 0, 2] = f["v_w2"][0:128, 0]
    wv2[:, 1, 2] = f["v_w2"][128:256, 0]

    shared = {
        "ones64": r(np.ones((BITS, 1), np.float32)),
        "ones1": r(np.ones((1, BITS), np.float32)),
        "wsd1": r(f["sd_w1"]),
        "wsd2": r(f["sd_w2"].reshape(4, 128, H).transpose(1, 0, 2)),
        "wsd3": r(f["sd_w3"].reshape(4, 128, BITS).transpose(1, 0, 2)),
        "sdb1": f["sd_b1"].reshape(4, 128).T,
        "sdb2": f["sd_b2"].reshape(4, 128).T,
        "sdb3": f["sd_b3"][:, None],
        "wixb": r(s1 * f["ix_w1"][BITS:]),
        "wvb": r(f["v_w1"][BITS:]),
        "wix2": to_f8(s2 * f["ix_w2"].reshape(4, 128, H).transpose(1, 0, 2)),
        "b2s": (s1 * s2) * f["ix_b2"].reshape(4, 128).T,
        "wix3": to_f8(s3 * f["ix_w3"].reshape(4, 128, BITS).transpose(1, 0, 2)),
        "ixb3": f["ix_b3"][:, None],
        "dn_w": dn.astype(ml_dtypes.bfloat16),
        "wv2x": wv2.astype(ml_dtypes.bfloat16),
        "ident8": np.eye(8, dtype=np.float32).astype(ml_dtypes.bfloat16),
        "vb2t": np.full((8, 1), 0.5 * float(f["v_b2"][0]), np.float32),
    }
    pbf = (f["ix_w1"][:BITS].T + f["ix_b1"][:, None]) * s1   # [512, 64]
    pbf = pbf.reshape(4, 128, BITS).transpose(1, 0, 2)       # [128, 4, 64]
    vpf = (f["v_w1"][:BITS].T + f["v_b1"][:, None])          # [256, 64]
    vpf = vpf.reshape(2, 128, BITS).transpose(1, 0, 2)       # [128, 2, 64]

    in_maps = []
    for c in range(NCORES):
        g, q = c // 4, c % 4
        sb = f["shift_bits"][q * BCC:(q + 1) * BCC]
        ab = f["a_bits"][q * BCC:(q + 1) * BCC]
        m = dict(shared)
        m["sbT"] = r(np.ascontiguousarray(sb.T))
        m["abT"] = np.ascontiguousarray(ab.T).astype(ml_dtypes.bfloat16)
        m["pb"] = np.ascontiguousarray(pbf[:, :, g * NPOS:(g + 1) * NPOS])
        m["vpb"] = np.ascontiguousarray(vpf[:, :, g * NPOS:(g + 1) * NPOS])
        mm = {}
        for k, v in m.items():
            if v.dtype == np.float64:
                v = v.astype(np.float32)
            mm[k] = np.ascontiguousarray(v)
        in_maps.append(mm)
    return in_maps


def run(inputs, trace=False):
    in_maps = make_in_maps(inputs)
    nc = _get_program()
    res = bass_utils.run_bass_kernel_spmd(
        nc, in_maps, core_ids=list(range(NCORES)), trace=trace)
    full = np.empty((B, BITS), np.float32)
    for c in range(NCORES):
        g, q = c // 4, c % 4
        full[q * BCC:(q + 1) * BCC, g * NPOS:(g + 1) * NPOS] = res.results[c]["out"]
    return full, res


def kernel(**inputs):
    return run(inputs)[0]


# revision 6
# speedup vs baseline: 1.2452x; 1.2452x over previous
"""Trainium2 Bass kernel for DecomposedShiftNet, v2 (fp8 DoubleRow).

Sharding: 8 cores = 4 batch quarters x 2 position halves.
  core = g*4 + q : batch rows q*512..q*512+512, positions g*32..g*32+32.
Per core: BCC=512 batch cols, 32 positions; block = 1 position x 512 batch.

Per block (NB=512 free cols):
  h1   : 4x tensor_scalar (sp_c + pb[c,i], max 0) -> fp8, scaled 2^SH1
  h2   : 8 fp8 DoubleRow matmuls (K=512 as 2 k-pairs x 4 m-chunks) into
         two [128,2,512] psum tiles; evict Relu+bias -> fp8 (scale folded)
  lg   : 2 fp8 DoubleRow matmuls (M=64, no dup); Exp evict (scale arg) ->
         exp_t[0:64] bf16; DVE mult by abT -> exp_t[64:128]
  dn/v : one [3,NB] psum: dn_w (M=3: 2*ones_d | ones_n | 0) on exp_t K=128
         + 2 bf16 matmuls (0|0|w2c) on vh K=2x128; copy evict + 1 DMA to
         stash[j] ([sn,3,NB] segment accumulators)
  vh   : 2x tensor_scalar bf16 (DVE 4x mode)
Tail per 8-block segment: rd=recip_approx_fast(2d), pq=n*rd (TT),
  tgh=Tanh(0.5v+0.5b) [same act table as Exp/Relu], out=(tgh+1)*pq (STT)
  -> bf16, PE transposes -> obm [128,4,32] f32 -> one DMA out [512,32].
"""

import sys

import ml_dtypes
import numpy as np

for _p in ("/opt/trn_rl_repo",):
    if _p not in sys.path:
        sys.path.insert(0, _p)

import concourse.bacc as bacc
import concourse.bass as bass
import concourse.tile as tile
from concourse import bass_utils, mybir

F32 = mybir.dt.float32
F32R = mybir.dt.float32r
BF16 = mybir.dt.bfloat16
F8 = mybir.dt.float8e4
AF = mybir.ActivationFunctionType
OP = mybir.AluOpType
DR = mybir.MatmulPerfMode.DoubleRow

B, BITS, H = 2048, 64, 512
NCORES = 8
BCC = 512           # batch rows per core
NPOS = 32           # positions per core
NB = BCC            # free columns per block (1 position)
HV = H // 2         # validity hidden = 256
SEGS = [(0, 8), (8, 8), (16, 8), (24, 4), (28, 4)]
SEG_OF = {}
for _si, (_s0, _sn) in enumerate(SEGS):
    for _k in range(_sn):
        SEG_OF[_s0 + _k] = (_si, _k)

# power-of-2 scales, computed in make_in_maps (module-level so _emit's
# compiled program is scale-agnostic: scales only touch host-side data
# except the Exp scale which is baked in -> keep it a fixed constant).
SH1 = 7    # h1 fp8 = 2^SH1 * h1_true  (h1 max ~0.1 -> 12.8)
SW2 = 4    # w2 fp8 = 2^SW2 * w2       (h2 scale 2^11: h2 max ~0.06 -> 123)
SW3 = 6    # w3 fp8 = 2^SW3 * w3
SH2 = SH1 + SW2
EXP_SCALE = 2.0 ** (-(SH2 + SW3))


def to_f32r_np(a):
    u = np.ascontiguousarray(a, dtype=np.float32).view(np.uint32)
    r = (u + 0x7FF + ((u >> 12) & 1)) & np.uint32(0xFFFFF000)
    return r.view(np.float32)


def to_f8(a):
    a = np.clip(np.asarray(a, np.float32), -224.0, 224.0)
    return a.astype(ml_dtypes.float8_e4m3fn)


# name -> (shape, dtype)
_INPUTS = {
    "sbT": ((BITS, BCC), "f32r"),
    "abT": ((BITS, BCC), "bf16"),
    "ones64": ((BITS, 1), "f32r"),
    "ones1": ((1, BITS), "f32r"),
    "pb": ((128, 4, NPOS), "f32"),
    "vpb": ((128, 2, NPOS), "f32"),
    "wsd1": ((BITS, H), "f32r"),
    "wsd2": ((128, 4, H), "f8"),
    "wsd3": ((128, 4, BITS), "f8"),
    "sdb1": ((128, 4), "f32"),
    "sdb2": ((128, 4), "f32"),
    "sdb3": ((BITS, 1), "f32"),
    "wixb": ((BITS, H), "f32r"),
    "wvb": ((BITS, HV), "f32r"),
    "wix2": ((128, 4, H), "f8"),
    "b2s": ((128, 4), "f32"),
    "wix3": ((128, 4, BITS), "f8"),
    "ixb3": ((BITS, 1), "f32"),
    "dn_w": ((128, 32), "bf16"),
    "wv2x": ((128, 2, 32), "bf16"),
    "ident8": ((8, 8), "bf16"),
    "vb2t": ((8, 1), "f32"),
}
_DT = {"f32": F32, "f32r": F32R, "bf16": BF16, "f8": F8}


def _emit(nc, tc, I, out):
    import contextlib

    ctx = contextlib.ExitStack()
    with ctx:
        const = ctx.enter_context(tc.tile_pool(name="const", bufs=1))
        work = ctx.enter_context(tc.tile_pool(name="work", bufs=5))
        psH = ctx.enter_context(tc.tile_pool(name="psH", bufs=2, space="PSUM"))
        psL = ctx.enter_context(tc.tile_pool(name="psL", bufs=2, space="PSUM"))
        psD = ctx.enter_context(tc.tile_pool(name="psD", bufs=2, space="PSUM"))

        # ---------------- load inputs (round-robin DMA queues) ----------------
        T = {}
        qs = [nc.sync, nc.gpsimd]

        def load(name, qi):
            shape, code = _INPUTS[name]
            t = const.tile(list(shape), _DT[code], tag=name, name=name)
            qs[qi % len(qs)].dma_start(out=t, in_=I[name])
            T[name] = t

        def load_split(name):
            shape, code = _INPUTS[name]
            t = const.tile(list(shape), _DT[code], tag=name, name=name)
            h = shape[-1] // 2
            nc.sync.dma_start(out=t[..., 0:h], in_=I[name][..., 0:h])
            nc.gpsimd.dma_start(out=t[..., h:], in_=I[name][..., h:])
            T[name] = t

        load_split("sbT")
        load_split("wsd1")
        order = ["sdb1", "wsd2", "sdb2", "wsd3", "sdb3",
                 "ones64", "ones1", "wixb", "pb", "wix2", "b2s", "abT",
                 "wvb", "vpb", "wix3", "ixb3", "dn_w", "wv2x", "ident8",
                 "vb2t"]
        for qi, name in enumerate(order):
            load(name, qi)

        # PE warm-up: ~3us of dummy matmuls during the input-DMA window so
        # the HAM clock gate reaches 8/8 before the shift-decoder chain runs.
        wu = const.tile([128, 64], BF16, tag="wu", name="wu")
        nc.gpsimd.memset(wu, 0.25)
        wups = psD.tile([64, 64], F32, tag="dnv", name="wups")
        for _ in range(28):
            nc.tensor.matmul(wups, wu[:, 0:64], wu, start=True, stop=True)

        sbT, abT = T["sbT"], T["abT"]
        ones64, ones1 = T["ones64"], T["ones1"]
        pb, vpb = T["pb"], T["vpb"]
        wsd1, wsd2, wsd3 = T["wsd1"], T["wsd2"], T["wsd3"]
        sdb1, sdb2, sdb3 = T["sdb1"], T["sdb2"], T["sdb3"]
        wixb, wvb = T["wixb"], T["wvb"]
        wix2, b2s = T["wix2"], T["b2s"]
        wix3, ixb3 = T["wix3"], T["ixb3"]
        dn_w, wv2x, ident8 = T["dn_w"], T["wv2x"], T["ident8"]

        # ---------------- shift decoder MLP (f32r, N=512) ----------------
        hsd1 = const.tile([128, 4, BCC], F8)
        for m in range(4):
            ps = psL.tile([128, BCC], F32, tag="lg", name="ps1")
            nc.tensor.matmul(ps, wsd1[:, m * 128:(m + 1) * 128], sbT,
                             start=True, stop=True)
            # hsd1 = 2^7 * relu(ps + b1); sdb1 host-scaled by 2^7
            nc.scalar.activation(hsd1[:, m, :], ps, AF.Relu,
                                 bias=sdb1[:, m:m + 1], scale=128.0)
        hsd2 = const.tile([128, 4, BCC], F8)
        for m in range(4):
            ps = psL.tile([128, BCC], F32, tag="lg", name="ps2")
            for kk in range(2):
                nc.tensor.matmul(ps, wsd2[:, 2 * kk:2 * kk + 2, m * 128:(m + 1) * 128],
                                 hsd1[:, 2 * kk:2 * kk + 2, :],
                                 start=(kk == 0), stop=(kk == 1), perf_mode=DR)
            # psum = 2^11 * x; hsd2 = 2^8 * relu(x + b2); sdb2 host-scaled 2^8
            nc.scalar.activation(hsd2[:, m, :], ps, AF.Relu,
                                 bias=sdb2[:, m:m + 1], scale=0.125)
        ps3 = psL.tile([64, BCC], F32, tag="lg", name="ps3")
        for kk in range(2):
            nc.tensor.matmul(ps3, wsd3[:, 2 * kk:2 * kk + 2, :],
                             hsd2[:, 2 * kk:2 * kk + 2, :],
                             start=(kk == 0), stop=(kk == 1), perf_mode=DR)
        exp_sd = const.tile([64, BCC], F32R)
        # psum = 2^14 * logits
        nc.scalar.activation(exp_sd, ps3, AF.Exp, bias=sdb3, scale=2.0 ** -14)

        # softmax normalize via ones-matmul + approx reciprocal + K=1 bcast
        psd = psD.tile([1, BCC], F32, tag="dnv", name="psd")
        nc.tensor.matmul(psd, ones64, exp_sd, start=True, stop=True)
        rec_sd = const.tile([1, BCC], F32)
        nc.vector.reciprocal_approx_fast(out=rec_sd, in_=psd)
        rec_r = const.tile([1, BCC], F32R)
        nc.vector.tensor_copy(out=rec_r, in_=rec_sd)
        psb = psL.tile([64, BCC], F32, tag="lg", name="psb")
        nc.tensor.matmul(psb, ones1, rec_r, start=True, stop=True)
        psb_s = const.tile([64, BCC], F32R)
        nc.vector.tensor_copy(out=psb_s, in_=psb)
        shift_soft = const.tile([64, BCC], F32R)
        nc.vector.tensor_tensor(shift_soft, exp_sd, psb_s, OP.mult)

        # sp = 2^SH1 * shift_soft @ ix_w1[64:]  (wixb pre-scaled), bf16
        sp = const.tile([128, 4, BCC], BF16)
        for m in range(4):
            ps = psL.tile([128, BCC], F32, tag="lg", name="psp")
            nc.tensor.matmul(ps, wixb[:, m * 128:(m + 1) * 128], shift_soft,
                             start=True, stop=True)
            if m % 2 == 0:
                nc.scalar.activation(sp[:, m, :], ps, AF.Copy)
            else:
                nc.vector.tensor_copy(out=sp[:, m, :], in_=ps)
        vs = const.tile([128, 2, BCC], BF16)
        for m in range(2):
            ps = psL.tile([128, BCC], F32, tag="lg", name="psv")
            nc.tensor.matmul(ps, wvb[:, m * 128:(m + 1) * 128], shift_soft,
                             start=True, stop=True)
            if m == 0:
                nc.scalar.activation(vs[:, m, :], ps, AF.Copy)
            else:
                nc.vector.tensor_copy(out=vs[:, m, :], in_=ps)

        # ---------------- stash accumulators + output staging ----------------
        stash = [const.tile([sn, 3, NB], F32, tag=f"stash{i}", name=f"stash{i}")
                 for i, (_, sn) in enumerate(SEGS)]
        obm = const.tile([128, 4, NPOS], F32, tag="obm", name="obm")

        st = {}

        def stage_h1_h2(j):
            d = st[j] = {}
            h1 = work.tile([128, 4, NB], F8, tag="h1", name="h1")
            for c in range(4):
                nc.vector.tensor_scalar(h1[:, c, :], sp[:, c, :],
                                        pb[:, c, j:j + 1], 0.0, OP.add, OP.max)
            # h2: 3-deep ring of [128, 2, NB] psum tiles; 2 DR k-pairs per m
            psa = psH.tile([128, 2, NB], F32, tag="h2", name="h2a")
            psc = psH.tile([128, 2, NB], F32, tag="h2", name="h2b")
            for m in range(4):
                pt = psa if m < 2 else psc
                for kk in range(2):
                    nc.tensor.matmul(
                        pt[:, m % 2, :],
                        wix2[:, 2 * kk:2 * kk + 2, m * 128:(m + 1) * 128],
                        h1[:, 2 * kk:2 * kk + 2, :],
                        start=(kk == 0), stop=(kk == 1), perf_mode=DR)
            h2 = d["h2"] = work.tile([128, 4, NB], F8, tag="h2", name="h2")
            nc.scalar.activation(h2[:, 0:2, :], psa, AF.Relu, bias=b2s[:, 0:1])
            nc.scalar.activation(h2[:, 2:4, :], psc, AF.Relu, bias=b2s[:, 2:3])
            vh = d["vh"] = work.tile([128, 2, NB], BF16, tag="vh", name="vh")
            for c in range(2):
                nc.vector.tensor_scalar(vh[:, c, :], vs[:, c, :],
                                        vpb[:, c, j:j + 1], 0.0, OP.add, OP.max)

        def stage_logits(j):
            d = st[j]
            pl = psL.tile([64, NB], F32, tag="lg", name="lg")
            for kk in range(2):
                nc.tensor.matmul(
                    pl, wix3[:, 2 * kk:2 * kk + 2, :],
                    d["h2"][:, 2 * kk:2 * kk + 2, :],
                    start=(kk == 0), stop=(kk == 1), perf_mode=DR)
            exp_t = d["exp"] = work.tile([128, NB], BF16, tag="exp", name="exp")
            nc.scalar.activation(exp_t[0:64, :], pl, AF.Exp,
                                 bias=ixb3, scale=EXP_SCALE)
            nc.vector.tensor_tensor(exp_t[64:128, :], exp_t[0:64, :], abT, OP.mult)

        grp = {}

        def stage_reduce(j):
            d = st[j]
            r = j % 4
            if r == 0:
                grp["pdn"] = psD.tile([128, NB], F32, tag="dnv", name="dnv")
            pdn = grp["pdn"]
            nc.tensor.matmul(pdn[32 * r:32 * r + 32, :], dn_w, d["exp"],
                             start=True, stop=False, tile_position=(0, 32 * r))
            for c in range(2):
                nc.tensor.matmul(pdn[32 * r:32 * r + 32, :], wv2x[:, c, :],
                                 d["vh"][:, c, :], start=False, stop=(c == 1),
                                 tile_position=(0, 32 * r))
            if r == 3:
                row = work.tile([128, NB], F32, tag="dnvsb", name="dnvsb")
                nc.vector.tensor_copy(out=row, in_=pdn)
                j0 = j - 3
                for g in range(4):
                    si, jj = SEG_OF[j0 + g]
                    nc.sync.dma_start(out=stash[si][jj:jj + 1, :, :],
                                      in_=row[32 * g:32 * g + 3, :])
            del st[j]

        tails = {}

        def tail_compute(si):
            s0, sn = SEGS[si]
            stv = stash[si]
            rd = work.tile([sn, NB], F32, tag="tailrd", name="tailrd")
            nc.vector.reciprocal_approx_fast(out=rd, in_=stv[:, 0, :])
            pq = work.tile([sn, NB], F32, tag="tailpq", name="tailpq")
            nc.vector.tensor_tensor(pq, stv[:, 1, :], rd, OP.mult)
            tgh = work.tile([sn, NB], F32, tag="tailtg", name="tailtg")
            nc.scalar.activation(tgh, stv[:, 2, :], AF.Tanh,
                                 bias=T["vb2t"][0:sn, :], scale=0.5)
            outv = tails[si] = work.tile([sn, NB], BF16, tag="tailout", name="tailout")
            nc.vector.scalar_tensor_tensor(outv, tgh, 1.0, pq, OP.add, OP.mult)

        outr = out.rearrange("(c p) j -> p c j", c=4)

        def tail_transpose(si):
            s0, sn = SEGS[si]
            outv = tails.pop(si)
            pt = psL.tile([128, 4, sn], BF16, tag="lg", name="ptr")
            for c in range(4):
                nc.tensor.transpose(pt[:, c, :], outv[:, c * 128:(c + 1) * 128],
                                    ident8[0:sn, 0:sn])
            nc.vector.tensor_copy(out=obm[:, :, s0:s0 + sn], in_=pt)
            nc.sync.dma_start(out=outr[:, :, s0:s0 + sn],
                              in_=obm[:, :, s0:s0 + sn])

        for j in range(NPOS):
            stage_h1_h2(j)
            if 4 <= j:
                stage_reduce(j - 4)
            if 2 <= j:
                stage_logits(j - 2)
            if j in (12, 20, 28):
                tail_compute((j - 12) // 8)
            if j in (14, 22, 30):
                tail_transpose((j - 14) // 8)
        stage_logits(NPOS - 2)
        stage_logits(NPOS - 1)
        for jj in range(NPOS - 4, NPOS):
            stage_reduce(jj)
        tail_compute(3)
        tail_transpose(3)
        tail_compute(4)
        tail_transpose(4)


def build_program():
    nc = bacc.Bacc("TRN2", target_bir_lowering=False, debug=False,
                   enable_asserts=False)
    I = {}
    for name, (shape, code) in _INPUTS.items():
        I[name] = nc.dram_tensor(name, list(shape), _DT[code],
                                 kind="ExternalInput").ap()
    out = nc.dram_tensor("out", [BCC, NPOS], F32, kind="ExternalOutput").ap()
    with tile.TileContext(nc) as tc:
        _emit(nc, tc, I, out)
    nc.compile()
    return nc


_NC = None


def _get_program():
    global _NC
    if _NC is None:
        _NC = build_program()
    return _NC


def make_in_maps(inputs):
    f = {k: np.ascontiguousarray(np.asarray(v, dtype=np.float32))
         for k, v in inputs.items()}
    r = to_f32r_np
    s1, s2, s3 = 2.0 ** SH1, 2.0 ** SW2, 2.0 ** SW3

    dn = np.zeros((128, 32), np.float32)
    dn[0:64, 0] = 2.0   # d*2 for the tail recip trick
    dn[64:128, 1] = 1.0
    wv2 = np.zeros((128, 2, 32), np.float32)
    wv2[:, 0, 2] = f["v_w2"][0:128, 0]
    wv2[:, 1, 2] = f["v_w2"][128:256, 0]

    shared = {
        "ones64": r(np.ones((BITS, 1), np.float32)),
        "ones1": r(np.ones((1, BITS), np.float32)),
        "wsd1": r(f["sd_w1"]),
        "wsd2": to_f8(16.0 * f["sd_w2"].reshape(4, 128, H).transpose(1, 0, 2)),
        "wsd3": to_f8(64.0 * f["sd_w3"].reshape(4, 128, BITS).transpose(1, 0, 2)),
        "sdb1": 128.0 * f["sd_b1"].reshape(4, 128).T,
        "sdb2": 256.0 * f["sd_b2"].reshape(4, 128).T,
        "sdb3": f["sd_b3"][:, None],
        "wixb": r(s1 * f["ix_w1"][BITS:]),
        "wvb": r(f["v_w1"][BITS:]),
        "wix2": to_f8(s2 * f["ix_w2"].reshape(4, 128, H).transpose(1, 0, 2)),
        "b2s": (s1 * s2) * f["ix_b2"].reshape(4, 128).T,
        "wix3": to_f8(s3 * f["ix_w3"].reshape(4, 128, BITS).transpose(1, 0, 2)),
        "ixb3": f["ix_b3"][:, None],
        "dn_w": dn.astype(ml_dtypes.bfloat16),
        "wv2x": wv2.astype(ml_dtypes.bfloat16),
        "ident8": np.eye(8, dtype=np.float32).astype(ml_dtypes.bfloat16),
        "vb2t": np.full((8, 1), 0.5 * float(f["v_b2"][0]), np.float32),
    }
    pbf = (f["ix_w1"][:BITS].T + f["ix_b1"][:, None]) * s1   # [512, 64]
    pbf = pbf.reshape(4, 128, BITS).transpose(1, 0, 2)       # [128, 4, 64]
    vpf = (f["v_w1"][:BITS].T + f["v_b1"][:, None])          # [256, 64]
    vpf = vpf.reshape(2, 128, BITS).transpose(1, 0, 2)       # [128, 2, 64]

    in_maps = []
    for c in range(NCORES):
        g, q = c // 4, c % 4
        sb = f["shift_bits"][q * BCC:(q + 1) * BCC]
        ab = f["a_bits"][q * BCC:(q + 1) * BCC]
        m = dict(shared)
        m["sbT"] = r(np.ascontiguousarray(sb.T))
        m["abT"] = np.ascontiguousarray(ab.T).astype(ml_dtypes.bfloat16)
        m["pb"] = np.ascontiguousarray(pbf[:, :, g * NPOS:(g + 1) * NPOS])
        m["vpb"] = np.ascontiguousarray(vpf[:, :, g * NPOS:(g + 1) * NPOS])
        mm = {}
        for k, v in m.items():
            if v.dtype == np.float64:
                v = v.astype(np.float32)
            mm[k] = np.ascontiguousarray(v)
        in_maps.append(mm)
    return in_maps


def run(inputs, trace=False):
    in_maps = make_in_maps(inputs)
    nc = _get_program()
    res = bass_utils.run_bass_kernel_spmd(
        nc, in_maps, core_ids=list(range(NCORES)), trace=trace)
    full = np.empty((B, BITS), np.float32)
    for c in range(NCORES):
        g, q = c // 4, c % 4
        full[q * BCC:(q + 1) * BCC, g * NPOS:(g + 1) * NPOS] = res.results[c]["out"]
    return full, res


def kernel(**inputs):
    return run(inputs)[0]


# revision 7
# speedup vs baseline: 1.2507x; 1.0044x over previous
"""Trainium2 Bass kernel for DecomposedShiftNet, v2 (fp8 DoubleRow).

Sharding: 8 cores = 4 batch quarters x 2 position halves.
  core = g*4 + q : batch rows q*512..q*512+512, positions g*32..g*32+32.
Per core: BCC=512 batch cols, 32 positions; block = 1 position x 512 batch.

Per block (NB=512 free cols):
  h1   : 4x tensor_scalar (sp_c + pb[c,i], max 0) -> fp8, scaled 2^SH1
  h2   : 8 fp8 DoubleRow matmuls (K=512 as 2 k-pairs x 4 m-chunks) into
         two [128,2,512] psum tiles; evict Relu+bias -> fp8 (scale folded)
  lg   : 2 fp8 DoubleRow matmuls (M=64, no dup); Exp evict (scale arg) ->
         exp_t[0:64] bf16; DVE mult by abT -> exp_t[64:128]
  dn/v : one [3,NB] psum: dn_w (M=3: 2*ones_d | ones_n | 0) on exp_t K=128
         + 2 bf16 matmuls (0|0|w2c) on vh K=2x128; copy evict + 1 DMA to
         stash[j] ([sn,3,NB] segment accumulators)
  vh   : 2x tensor_scalar bf16 (DVE 4x mode)
Tail per 8-block segment: rd=recip_approx_fast(2d), pq=n*rd (TT),
  tgh=Tanh(0.5v+0.5b) [same act table as Exp/Relu], out=(tgh+1)*pq (STT)
  -> bf16, PE transposes -> obm [128,4,32] f32 -> one DMA out [512,32].
"""

import sys

import ml_dtypes
import numpy as np

for _p in ("/opt/trn_rl_repo",):
    if _p not in sys.path:
        sys.path.insert(0, _p)

import concourse.bacc as bacc
import concourse.bass as bass
import concourse.tile as tile
from concourse import bass_utils, mybir

F32 = mybir.dt.float32
F32R = mybir.dt.float32r
BF16 = mybir.dt.bfloat16
F8 = mybir.dt.float8e4
AF = mybir.ActivationFunctionType
OP = mybir.AluOpType
DR = mybir.MatmulPerfMode.DoubleRow

B, BITS, H = 2048, 64, 512
NCORES = 8
BCC = 512           # batch rows per core
NPOS = 32           # positions per core
NB = BCC            # free columns per block (1 position)
HV = H // 2         # validity hidden = 256
SEGS = [(0, 8), (8, 8), (16, 8), (24, 4), (28, 4)]
SEG_OF = {}
for _si, (_s0, _sn) in enumerate(SEGS):
    for _k in range(_sn):
        SEG_OF[_s0 + _k] = (_si, _k)

# power-of-2 scales, computed in make_in_maps (module-level so _emit's
# compiled program is scale-agnostic: scales only touch host-side data
# except the Exp scale which is baked in -> keep it a fixed constant).
SH1 = 7    # h1 fp8 = 2^SH1 * h1_true  (h1 max ~0.1 -> 12.8)
SW2 = 4    # w2 fp8 = 2^SW2 * w2       (h2 scale 2^11: h2 max ~0.06 -> 123)
SW3 = 6    # w3 fp8 = 2^SW3 * w3
SH2 = SH1 + SW2
EXP_SCALE = 2.0 ** (-(SH2 + SW3))


def to_f32r_np(a):
    u = np.ascontiguousarray(a, dtype=np.float32).view(np.uint32)
    r = (u + 0x7FF + ((u >> 12) & 1)) & np.uint32(0xFFFFF000)
    return r.view(np.float32)


def to_f8(a):
    a = np.clip(np.asarray(a, np.float32), -224.0, 224.0)
    return a.astype(ml_dtypes.float8_e4m3fn)


# name -> (shape, dtype)
_INPUTS = {
    "sbT": ((BITS, BCC), "f32r"),
    "abT": ((BITS, BCC), "bf16"),
    "ones64": ((BITS, 1), "f32r"),
    "ones1": ((1, BITS), "f32r"),
    "pb": ((128, 4, NPOS), "f32"),
    "vpb": ((128, 2, NPOS), "f32"),
    "wsd1": ((BITS, H), "f32r"),
    "wsd2": ((128, 4, H), "f8"),
    "wsd3": ((128, 4, BITS), "f8"),
    "sdb1": ((128, 4), "f32"),
    "sdb2": ((128, 4), "f32"),
    "sdb3": ((BITS, 1), "f32"),
    "wixb": ((BITS, H), "f32r"),
    "wvb": ((BITS, HV), "f32r"),
    "wix2": ((128, 4, H), "f8"),
    "b2s": ((128, 4), "f32"),
    "wix3": ((128, 4, BITS), "f8"),
    "ixb3": ((BITS, 1), "f32"),
    "dn_w": ((128, 32), "bf16"),
    "wv2x": ((128, 2, 32), "bf16"),
    "ident8": ((8, 8), "bf16"),
    "vb2t": ((8, 1), "f32"),
}
_DT = {"f32": F32, "f32r": F32R, "bf16": BF16, "f8": F8}


def _emit(nc, tc, I, out):
    import contextlib

    ctx = contextlib.ExitStack()
    with ctx:
        const = ctx.enter_context(tc.tile_pool(name="const", bufs=1))
        work = ctx.enter_context(tc.tile_pool(name="work", bufs=8))
        psH = ctx.enter_context(tc.tile_pool(name="psH", bufs=2, space="PSUM"))
        psL = ctx.enter_context(tc.tile_pool(name="psL", bufs=2, space="PSUM"))
        psD = ctx.enter_context(tc.tile_pool(name="psD", bufs=2, space="PSUM"))

        # ---------------- load inputs (round-robin DMA queues) ----------------
        T = {}
        qs = [nc.sync, nc.gpsimd]

        def load(name, qi):
            shape, code = _INPUTS[name]
            t = const.tile(list(shape), _DT[code], tag=name, name=name)
            qs[qi % len(qs)].dma_start(out=t, in_=I[name])
            T[name] = t

        def load_split(name):
            shape, code = _INPUTS[name]
            t = const.tile(list(shape), _DT[code], tag=name, name=name)
            h = shape[-1] // 2
            nc.sync.dma_start(out=t[..., 0:h], in_=I[name][..., 0:h])
            nc.gpsimd.dma_start(out=t[..., h:], in_=I[name][..., h:])
            T[name] = t

        load_split("sbT")
        load_split("wsd1")
        order = ["sdb1", "wsd2", "sdb2", "wsd3", "sdb3",
                 "ones64", "ones1", "wixb", "pb", "wix2", "b2s", "abT",
                 "wvb", "vpb", "wix3", "ixb3", "dn_w", "wv2x", "ident8",
                 "vb2t"]
        for qi, name in enumerate(order):
            load(name, qi)

        # PE warm-up: ~3us of dummy matmuls during the input-DMA window so
        # the HAM clock gate reaches 8/8 before the shift-decoder chain runs.
        wu = const.tile([128, 64], BF16, tag="wu", name="wu")
        nc.gpsimd.memset(wu, 0.25)
        wups = psD.tile([64, 64], F32, tag="dnv", name="wups")
        for _ in range(28):
            nc.tensor.matmul(wups, wu[:, 0:64], wu, start=True, stop=True)

        sbT, abT = T["sbT"], T["abT"]
        ones64, ones1 = T["ones64"], T["ones1"]
        pb, vpb = T["pb"], T["vpb"]
        wsd1, wsd2, wsd3 = T["wsd1"], T["wsd2"], T["wsd3"]
        sdb1, sdb2, sdb3 = T["sdb1"], T["sdb2"], T["sdb3"]
        wixb, wvb = T["wixb"], T["wvb"]
        wix2, b2s = T["wix2"], T["b2s"]
        wix3, ixb3 = T["wix3"], T["ixb3"]
        dn_w, wv2x, ident8 = T["dn_w"], T["wv2x"], T["ident8"]

        # ---------------- shift decoder MLP (f32r, N=512) ----------------
        hsd1 = const.tile([128, 4, BCC], F8)
        for m in range(4):
            ps = psL.tile([128, BCC], F32, tag="lg", name="ps1")
            nc.tensor.matmul(ps, wsd1[:, m * 128:(m + 1) * 128], sbT,
                             start=True, stop=True)
            # hsd1 = 2^7 * relu(ps + b1); sdb1 host-scaled by 2^7
            nc.scalar.activation(hsd1[:, m, :], ps, AF.Relu,
                                 bias=sdb1[:, m:m + 1], scale=128.0)
        hsd2 = const.tile([128, 4, BCC], F8)
        for m in range(4):
            ps = psL.tile([128, BCC], F32, tag="lg", name="ps2")
            for kk in range(2):
                nc.tensor.matmul(ps, wsd2[:, 2 * kk:2 * kk + 2, m * 128:(m + 1) * 128],
                                 hsd1[:, 2 * kk:2 * kk + 2, :],
                                 start=(kk == 0), stop=(kk == 1), perf_mode=DR)
            # psum = 2^11 * x; hsd2 = 2^8 * relu(x + b2); sdb2 host-scaled 2^8
            nc.scalar.activation(hsd2[:, m, :], ps, AF.Relu,
                                 bias=sdb2[:, m:m + 1], scale=0.125)
        ps3 = psL.tile([64, BCC], F32, tag="lg", name="ps3")
        for kk in range(2):
            nc.tensor.matmul(ps3, wsd3[:, 2 * kk:2 * kk + 2, :],
                             hsd2[:, 2 * kk:2 * kk + 2, :],
                             start=(kk == 0), stop=(kk == 1), perf_mode=DR)
        exp_sd = const.tile([64, BCC], F32R)
        # psum = 2^14 * logits
        nc.scalar.activation(exp_sd, ps3, AF.Exp, bias=sdb3, scale=2.0 ** -14)

        # softmax normalize via ones-matmul + approx reciprocal + K=1 bcast
        psd = psD.tile([1, BCC], F32, tag="dnv", name="psd")
        nc.tensor.matmul(psd, ones64, exp_sd, start=True, stop=True)
        rec_sd = const.tile([1, BCC], F32)
        nc.vector.reciprocal_approx_fast(out=rec_sd, in_=psd)
        rec_r = const.tile([1, BCC], F32R)
        nc.vector.tensor_copy(out=rec_r, in_=rec_sd)
        psb = psL.tile([64, BCC], F32, tag="lg", name="psb")
        nc.tensor.matmul(psb, ones1, rec_r, start=True, stop=True)
        psb_s = const.tile([64, BCC], F32R)
        nc.vector.tensor_copy(out=psb_s, in_=psb)
        shift_soft = const.tile([64, BCC], F32R)
        nc.vector.tensor_tensor(shift_soft, exp_sd, psb_s, OP.mult)

        # sp = 2^SH1 * shift_soft @ ix_w1[64:]  (wixb pre-scaled), bf16
        sp = const.tile([128, 4, BCC], BF16)
        for m in range(4):
            ps = psL.tile([128, BCC], F32, tag="lg", name="psp")
            nc.tensor.matmul(ps, wixb[:, m * 128:(m + 1) * 128], shift_soft,
                             start=True, stop=True)
            if m % 2 == 0:
                nc.scalar.activation(sp[:, m, :], ps, AF.Copy)
            else:
                nc.vector.tensor_copy(out=sp[:, m, :], in_=ps)
        vs = const.tile([128, 2, BCC], BF16)
        for m in range(2):
            ps = psL.tile([128, BCC], F32, tag="lg", name="psv")
            nc.tensor.matmul(ps, wvb[:, m * 128:(m + 1) * 128], shift_soft,
                             start=True, stop=True)
            if m == 0:
                nc.scalar.activation(vs[:, m, :], ps, AF.Copy)
            else:
                nc.vector.tensor_copy(out=vs[:, m, :], in_=ps)

        # ---------------- stash accumulators + output staging ----------------
        stash = [const.tile([sn, 3, NB], F32, tag=f"stash{i}", name=f"stash{i}")
                 for i, (_, sn) in enumerate(SEGS)]
        obm = const.tile([128, 4, NPOS], F32, tag="obm", name="obm")

        st = {}

        def stage_h1_h2(j):
            d = st[j] = {}
            h1 = work.tile([128, 4, NB], F8, tag="h1", name="h1")
            for c in range(4):
                nc.vector.tensor_scalar(h1[:, c, :], sp[:, c, :],
                                        pb[:, c, j:j + 1], 0.0, OP.add, OP.max)
            # h2: 3-deep ring of [128, 2, NB] psum tiles; 2 DR k-pairs per m
            psa = psH.tile([128, 2, NB], F32, tag="h2", name="h2a")
            psc = psH.tile([128, 2, NB], F32, tag="h2", name="h2b")
            for m in range(4):
                pt = psa if m < 2 else psc
                for kk in range(2):
                    nc.tensor.matmul(
                        pt[:, m % 2, :],
                        wix2[:, 2 * kk:2 * kk + 2, m * 128:(m + 1) * 128],
                        h1[:, 2 * kk:2 * kk + 2, :],
                        start=(kk == 0), stop=(kk == 1), perf_mode=DR)
            h2 = d["h2"] = work.tile([128, 4, NB], F8, tag="h2", name="h2")
            nc.scalar.activation(h2[:, 0:2, :], psa, AF.Relu, bias=b2s[:, 0:1])
            nc.scalar.activation(h2[:, 2:4, :], psc, AF.Relu, bias=b2s[:, 2:3])
            vh = d["vh"] = work.tile([128, 2, NB], BF16, tag="vh", name="vh")
            for c in range(2):
                nc.vector.tensor_scalar(vh[:, c, :], vs[:, c, :],
                                        vpb[:, c, j:j + 1], 0.0, OP.add, OP.max)

        def stage_logits(j):
            d = st[j]
            pl = psL.tile([64, NB], F32, tag="lg", name="lg")
            for kk in range(2):
                nc.tensor.matmul(
                    pl, wix3[:, 2 * kk:2 * kk + 2, :],
                    d["h2"][:, 2 * kk:2 * kk + 2, :],
                    start=(kk == 0), stop=(kk == 1), perf_mode=DR)
            exp_t = d["exp"] = work.tile([128, NB], BF16, tag="exp", name="exp")
            nc.scalar.activation(exp_t[0:64, :], pl, AF.Exp,
                                 bias=ixb3, scale=EXP_SCALE)
            nc.vector.tensor_tensor(exp_t[64:128, :], exp_t[0:64, :], abT, OP.mult)

        grp = {}

        def stage_reduce(j):
            d = st[j]
            r = j % 4
            if r == 0:
                grp["pdn"] = psD.tile([128, NB], F32, tag="dnv", name="dnv")
            pdn = grp["pdn"]
            nc.tensor.matmul(pdn[32 * r:32 * r + 32, :], dn_w, d["exp"],
                             start=True, stop=False, tile_position=(0, 32 * r))
            for c in range(2):
                nc.tensor.matmul(pdn[32 * r:32 * r + 32, :], wv2x[:, c, :],
                                 d["vh"][:, c, :], start=False, stop=(c == 1),
                                 tile_position=(0, 32 * r))
            if r == 3:
                row = work.tile([128, NB], F32, tag="dnvsb", name="dnvsb")
                nc.vector.tensor_copy(out=row, in_=pdn)
                j0 = j - 3
                for g in range(4):
                    si, jj = SEG_OF[j0 + g]
                    nc.sync.dma_start(out=stash[si][jj:jj + 1, :, :],
                                      in_=row[32 * g:32 * g + 3, :])
            del st[j]

        tails = {}

        def tail_compute(si):
            s0, sn = SEGS[si]
            stv = stash[si]
            rd = work.tile([sn, NB], F32, tag="tailrd", name="tailrd")
            nc.vector.reciprocal_approx_fast(out=rd, in_=stv[:, 0, :])
            pq = work.tile([sn, NB], F32, tag="tailpq", name="tailpq")
            nc.vector.tensor_tensor(pq, stv[:, 1, :], rd, OP.mult)
            tgh = work.tile([sn, NB], F32, tag="tailtg", name="tailtg")
            nc.scalar.activation(tgh, stv[:, 2, :], AF.Tanh,
                                 bias=T["vb2t"][0:sn, :], scale=0.5)
            outv = tails[si] = work.tile([sn, NB], BF16, tag="tailout", name="tailout")
            nc.vector.scalar_tensor_tensor(outv, tgh, 1.0, pq, OP.add, OP.mult)

        outr = out.rearrange("(c p) j -> p c j", c=4)

        def tail_transpose(si):
            s0, sn = SEGS[si]
            outv = tails.pop(si)
            pt = psL.tile([128, 4, sn], BF16, tag="lg", name="ptr")
            for c in range(4):
                nc.tensor.transpose(pt[:, c, :], outv[:, c * 128:(c + 1) * 128],
                                    ident8[0:sn, 0:sn])
            nc.vector.tensor_copy(out=obm[:, :, s0:s0 + sn], in_=pt)
            nc.sync.dma_start(out=outr[:, :, s0:s0 + sn],
                              in_=obm[:, :, s0:s0 + sn])

        for j in range(NPOS):
            stage_h1_h2(j)
            if 4 <= j:
                stage_reduce(j - 4)
            if 2 <= j:
                stage_logits(j - 2)
            if j in (12, 20, 28):
                tail_compute((j - 12) // 8)
            if j in (14, 22, 30):
                tail_transpose((j - 14) // 8)
        stage_logits(NPOS - 2)
        stage_logits(NPOS - 1)
        for jj in range(NPOS - 4, NPOS):
            stage_reduce(jj)
        tail_compute(3)
        tail_transpose(3)
        tail_compute(4)
        tail_transpose(4)


def build_program():
    nc = bacc.Bacc("TRN2", target_bir_lowering=False, debug=False,
                   enable_asserts=False)
    I = {}
    for name, (shape, code) in _INPUTS.items():
        I[name] = nc.dram_tensor(name, list(shape), _DT[code],
                                 kind="ExternalInput").ap()
    out = nc.dram_tensor("out", [BCC, NPOS], F32, kind="ExternalOutput").ap()
    with tile.TileContext(nc) as tc:
        _emit(nc, tc, I, out)
    nc.compile()
    return nc


_NC = None


def _get_program():
    global _NC
    if _NC is None:
        _NC = build_program()
    return _NC


def make_in_maps(inputs):
    f = {k: np.ascontiguousarray(np.asarray(v, dtype=np.float32))
         for k, v in inputs.items()}
    r = to_f32r_np
    s1, s2, s3 = 2.0 ** SH1, 2.0 ** SW2, 2.0 ** SW3

    dn = np.zeros((128, 32), np.float32)
    dn[0:64, 0] = 2.0   # d*2 for the tail recip trick
    dn[64:128, 1] = 1.0
    wv2 = np.zeros((128, 2, 32), np.float32)
    wv2[:, 0, 2] = f["v_w2"][0:128, 0]
    wv2[:, 1, 2] = f["v_w2"][128:256, 0]

    shared = {
        "ones64": r(np.ones((BITS, 1), np.float32)),
        "ones1": r(np.ones((1, BITS), np.float32)),
        "wsd1": r(f["sd_w1"]),
        "wsd2": to_f8(16.0 * f["sd_w2"].reshape(4, 128, H).transpose(1, 0, 2)),
        "wsd3": to_f8(64.0 * f["sd_w3"].reshape(4, 128, BITS).transpose(1, 0, 2)),
        "sdb1": 128.0 * f["sd_b1"].reshape(4, 128).T,
        "sdb2": 256.0 * f["sd_b2"].reshape(4, 128).T,
        "sdb3": f["sd_b3"][:, None],
        "wixb": r(s1 * f["ix_w1"][BITS:]),
        "wvb": r(f["v_w1"][BITS:]),
        "wix2": to_f8(s2 * f["ix_w2"].reshape(4, 128, H).transpose(1, 0, 2)),
        "b2s": (s1 * s2) * f["ix_b2"].reshape(4, 128).T,
        "wix3": to_f8(s3 * f["ix_w3"].reshape(4, 128, BITS).transpose(1, 0, 2)),
        "ixb3": f["ix_b3"][:, None],
        "dn_w": dn.astype(ml_dtypes.bfloat16),
        "wv2x": wv2.astype(ml_dtypes.bfloat16),
        "ident8": np.eye(8, dtype=np.float32).astype(ml_dtypes.bfloat16),
        "vb2t": np.full((8, 1), 0.5 * float(f["v_b2"][0]), np.float32),
    }
    pbf = (f["ix_w1"][:BITS].T + f["ix_b1"][:, None]) * s1   # [512, 64]
    pbf = pbf.reshape(4, 128, BITS).transpose(1, 0, 2)       # [128, 4, 64]
    vpf = (f["v_w1"][:BITS].T + f["v_b1"][:, None])          # [256, 64]
    vpf = vpf.reshape(2, 128, BITS).transpose(1, 0, 2)       # [128, 2, 64]

    in_maps = []
    for c in range(NCORES):
        g, q = c // 4, c % 4
        sb = f["shift_bits"][q * BCC:(q + 1) * BCC]
        ab = f["a_bits"][q * BCC:(q + 1) * BCC]
        m = dict(shared)
        m["sbT"] = r(np.ascontiguousarray(sb.T))
        m["abT"] = np.ascontiguousarray(ab.T).astype(ml_dtypes.bfloat16)
        m["pb"] = np.ascontiguousarray(pbf[:, :, g * NPOS:(g + 1) * NPOS])
        m["vpb"] = np.ascontiguousarray(vpf[:, :, g * NPOS:(g + 1) * NPOS])
        mm = {}
        for k, v in m.items():
            if v.dtype == np.float64:
                v = v.astype(np.float32)
            mm[k] = np.ascontiguousarray(v)
        in_maps.append(mm)
    return in_maps


def run(inputs, trace=False):
    in_maps = make_in_maps(inputs)
    nc = _get_program()
    res = bass_utils.run_bass_kernel_spmd(
        nc, in_maps, core_ids=list(range(NCORES)), trace=trace)
    full = np.empty((B, BITS), np.float32)
    for c in range(NCORES):
        g, q = c // 4, c % 4
        full[q * BCC:(q + 1) * BCC, g * NPOS:(g + 1) * NPOS] = res.results[c]["out"]
    return full, res


def kernel(**inputs):
    return run(inputs)[0]
